# revision 28
# baseline (speedup 1.0000x reference)
"""Trainium2 Bass kernel for nn_AttnLayer (additive-attention pooling layer).

Reference computation (per batch b):
    e = e_hiddens @ We_w.T + We_b            # [S, F]
    d = Wd_w @ d_hiddens[b]                  # [F]
    h = tanh(d + e)                          # [S, F]
    s = h @ v_w[0] + v_b                     # [S]
    a = softmax(s)                           # [S]
    out[b] = a @ e_hiddens[b]                # [D]

Strategy (8 cores, data-parallel over batch B=32 -> 4 per core):
  x pre-transposed ON HOST to [d-partition, s-free] bf16, processed in UNITS
  of 1-2 1024-long s-chunks of one batch (one exp + one big DVE mult per
  unit).  Batch 0's first two chunks are single-chunk units so the pipeline
  ramps while the DMA still streams.  Per unit:
    PE : e^T[f,s] = sum_k wet^T @ xt  (bf16); scores via a v-replicated
         stationary -> [128, s] replicated in PSUM
    ACT: h = tanh(e^T + d_b)  ;  a = exp(sc + v_b) over the whole unit in
         one op (accum_out -> unit partial of softmax Z)
    weighted sum x*a:
      DVE: ONE broadcast tensor_tensor (2x bf16 mode) multiplies all 8
           k-slices; slices 0..3 are reduced by an IN-PLACE bf16 halving
           fold tree (also 2x)
      Pool: the final 128->64 fold writes tails into a per-batch tile
      ACT: unit-wide copy+accum reduces for slices 4..7
  Per batch: DVE tensor_reduce folds the partial tails + ACT accum columns
  into [128, 8] per-k sums; host reorders [p,k]->[d] and divides by Z.
  A dummy e->tanh->scores->exp chain plus bf16 warmup matmuls run while x_0
  streams in, opening the PE clock gate and filling every engine pipeline.

Schedule: each engine's FIFO sees ops only after their producers ran one
iteration earlier (exp of unit u-2, tanh/scores of u-1, e of u, weighted sum
of u-2, ACT reduces of u-3), so no head-of-line blocking."""

import numpy as np
import ml_dtypes

import concourse.bass as bass
import concourse.bacc as bacc
import concourse.mybir as mybir
import concourse.tile as tile
from concourse.bass_utils import run_bass_kernel_spmd

F32 = mybir.dt.float32
BF16 = mybir.dt.bfloat16
AF = mybir.ActivationFunctionType
ALU = mybir.AluOpType
AX = mybir.AxisListType

N_CORES = 8
B, S, D, F = 32, 4096, 1024, 128
BP = B // N_CORES          # batches per core
KD = D // 128              # d-slices (partition groups)
SC = 1024                  # s-chunk
NCH = S // SC              # chunks per batch
NP = BP * (NCH // 2)       # record-pairs in DRAM layout

NKF = 4                    # k-slices reduced by the DVE fold tree
TW = 64                    # fold-tail width

# units: (dram_pair, r0, nr, batch, c0, pta_col).  Batch 0's first two
# chunks are single-record units to shorten the pipeline ramp.
UNITS = [(0, 0, 1, 0, 0, 0), (0, 1, 1, 0, 1, 1), (1, 0, 2, 0, 2, 2)] + [
    (p, 0, 2, p // 2, 2 * (p % 2), p % 2) for p in range(2, NP)]
NU = len(UNITS)
LAST_UNIT_OF_BATCH = {u[3]: i for i, u in enumerate(UNITS)}
ZCOL0_OF_BATCH = {0: (0, 3)}
for _i, _u in enumerate(UNITS):
    if _u[3] > 0:
        lo, hi = ZCOL0_OF_BATCH.get(_u[3], (_i, _i))
        ZCOL0_OF_BATCH[_u[3]] = (min(lo, _i), _i + 1)


def build_nc(bp=BP, s=S, d=D, f=F):
    nc = bacc.Bacc("TRN2", target_bir_lowering=False, debug=False)

    xt_dram = nc.dram_tensor("xt", [NP, 128, 2 * KD * SC], BF16,
                             kind="ExternalInput").ap()
    wet_dram = nc.dram_tensor("wet", [128, KD * f], BF16, kind="ExternalInput").ap()
    vrep_dram = nc.dram_tensor("vrep", [f, 128], BF16, kind="ExternalInput").ap()
    vbb_dram = nc.dram_tensor("vbb", [128, 1], F32, kind="ExternalInput").ap()
    dvec_dram = nc.dram_tensor("dvec", [f, bp], F32, kind="ExternalInput").ap()
    out_dram = nc.dram_tensor("out", [bp, 128, KD], F32, kind="ExternalOutput").ap()
    z_dram = nc.dram_tensor("z", [1, bp], F32, kind="ExternalOutput").ap()

    with tile.TileContext(nc) as tc:
        with (
            tc.tile_pool(name="const", bufs=1) as const,
            tc.tile_pool(name="xpool", bufs=3) as xpool,
            tc.tile_pool(name="mpool", bufs=2) as mpool,
            tc.tile_pool(name="hpool", bufs=3) as hpool,
            tc.tile_pool(name="apool", bufs=2) as apool,
            tc.tile_pool(name="ppool", bufs=2) as ppool,
            tc.tile_pool(name="ptpool", bufs=2) as ptpool,
            tc.tile_pool(name="opool", bufs=2) as opool,
            tc.tile_pool(name="ps_e", bufs=2, space="PSUM") as ps_e,
            tc.tile_pool(name="ps_sc", bufs=1, space="PSUM") as ps_sc,
        ):
            # ---- constants on parallel DMA queues (x-loads use sync) ----
            vrep_sb = const.tile([f, 128], BF16)
            nc.scalar.dma_start(vrep_sb, vrep_dram)
            wet_sb = const.tile([128, KD, f], BF16)
            nc.scalar.dma_start(wet_sb, wet_dram.rearrange("p (k f) -> p k f", k=KD))
            vbb_sb = const.tile([128, 1], F32)
            nc.gpsimd.dma_start(vbb_sb, vbb_dram)
            dvec_sb = const.tile([f, bp], F32)
            nc.gpsimd.dma_start(dvec_sb, dvec_dram)
            zcols_sb = const.tile([128, NU + 1], F32)
            zvals_sb = const.tile([1, bp], F32)

            def load_x(u):
                pair, r0, nr, _, _, _ = UNITS[u]
                xt = xpool.tile([128, 2, KD, SC], BF16, tag="x", name=f"x_{u}")
                src = xt_dram[pair].rearrange("p (r k s) -> p r k s", r=2, k=KD)
                for rr in range(nr):
                    nc.sync.dma_start(xt[:, rr], src[:, r0 + rr])
                return xt

            xts = {u: load_x(u) for u in range(min(3, NU))}

            state = {}
            pts = {}
            ptas = {}

            def get_pt(b):
                if b not in pts:
                    pts[b] = ptpool.tile([128, NKF, NCH, TW], BF16, tag="pt",
                                         name=f"pt_{b}")
                    ptas[b] = ptpool.tile([128, KD - NKF, 3], F32, tag="pta",
                                          name=f"pta_{b}")
                    if b > 0:
                        nc.vector.memset(ptas[b][:, :, 2:3], 0.0)
                return pts[b], ptas[b]

            def e_unit(u):
                nr = UNITS[u][2]
                xt = xts[u]
                es = []
                for rr in range(nr):
                    e_ps = ps_e.tile([f, SC], F32, tag="e", name=f"e_{u}_{rr}")
                    for k in range(KD):
                        for h2 in range(2):
                            sl = slice(h2 * 512, (h2 + 1) * 512)
                            nc.tensor.matmul(
                                e_ps[:, sl], wet_sb[:, k, :], xt[:, rr, k, sl],
                                start=(k == 0), stop=(k == KD - 1))
                    es.append(e_ps)
                state[u] = {"xt": xt, "e": es}

            def tanh_unit(u):
                b = UNITS[u][3]
                es = state[u].pop("e")
                hs = []
                for rr, e_ps in enumerate(es):
                    h_sb = hpool.tile([f, SC], BF16, tag="h", name=f"h_{u}_{rr}")
                    nc.scalar.activation(h_sb, e_ps, AF.Tanh,
                                         bias=dvec_sb[:, b:b + 1])
                    hs.append(h_sb)
                state[u]["h"] = hs

            def scores_unit(u):
                hs = state[u].pop("h")
                sc2 = ps_sc.tile([128, 2, SC], F32, tag="sc", name=f"sc_{u}")
                for rr, h_sb in enumerate(hs):
                    for h2 in range(2):
                        sl = slice(h2 * 512, (h2 + 1) * 512)
                        nc.tensor.matmul(sc2[:, rr, sl], vrep_sb, h_sb[:, sl],
                                         start=True, stop=True)
                state[u]["sc2"] = sc2

            def exp_unit(u):
                nr = UNITS[u][2]
                sc2 = state[u].pop("sc2")
                a2 = apool.tile([128, 2, SC], BF16, tag="a", name=f"a_{u}")
                nc.scalar.activation(
                    a2[:, 0:nr].rearrange("p r s -> p (r s)"),
                    sc2[:, 0:nr].rearrange("p r s -> p (r s)"),
                    AF.Exp, bias=vbb_sb, accum_out=zcols_sb[:, u:u + 1])
                state[u]["a2"] = a2

            def wsum_unit(u):
                _, _, nr, b, c0, _ = UNITS[u]
                st = state[u]
                xt, a2 = st.pop("xt"), st.pop("a2")
                xts.pop(u)
                pt, _ = get_pt(b)
                # DVE: one broadcast mult for all 8 k-slices of the unit
                # (last unit: ACT-reduced slices first so the drain overlaps)
                m2 = mpool.tile([128, 2, KD, SC], BF16, tag="m", name=f"m_{u}")
                ksls = ([slice(NKF, KD), slice(0, NKF)] if u == NU - 1
                        else [slice(0, KD)])
                for ksl in ksls:
                    nc.vector.tensor_tensor(
                        m2[:, 0:nr, ksl, :], xt[:, 0:nr, ksl, :],
                        a2[:, 0:nr].unsqueeze(2).broadcast_to(
                            [128, nr, ksl.stop - ksl.start, SC]),
                        op=ALU.mult)
                # DVE: in-place fold tree on slices 0..NKF-1 down to 2*TW
                h = SC // 2
                while h >= 2 * TW:
                    nc.vector.tensor_tensor(
                        m2[:, 0:nr, 0:NKF, 0:h], m2[:, 0:nr, 0:NKF, 0:h],
                        m2[:, 0:nr, 0:NKF, h:2 * h], op=ALU.add)
                    h //= 2
                # Pool: last fold drops tails into the per-batch partial tile
                nc.gpsimd.tensor_tensor(
                    pt[:, :, c0:c0 + nr, :],
                    m2[:, 0:nr, 0:NKF, 0:TW].rearrange("p r k s -> p k r s"),
                    m2[:, 0:nr, 0:NKF, TW:2 * TW].rearrange("p r k s -> p k r s"),
                    op=ALU.add)
                st["m2"] = m2

            def act_reduces(u):
                _, _, nr, b, _, pcol = UNITS[u]
                m2 = state.pop(u)["m2"]
                _, pta = get_pt(b)
                # ACT: unit-wide copy+accum reduces for k=NKF..7 (chunks of
                # one batch summed together; fin() sums chunks anyway)
                da = ppool.tile([128, 2, SC], BF16, tag="da", name=f"da_{u}")
                for ki in range(NKF, KD):
                    nc.scalar.activation(
                        da[:, 0:nr], m2[:, 0:nr, ki, :], AF.Copy,
                        accum_out=pta[:, ki - NKF, pcol:pcol + 1])

            def fin(b):
                pt, pta = pts.pop(b), ptas.pop(b)
                acc = opool.tile([128, KD], F32, tag="acc", name=f"acc_{b}")
                nc.vector.tensor_reduce(
                    acc[:, 0:NKF].unsqueeze(2),
                    pt.rearrange("p k c s -> p k (c s)"), axis=AX.X, op=ALU.add)
                nc.vector.tensor_reduce(
                    acc[:, NKF:KD].unsqueeze(2), pta, axis=AX.X, op=ALU.add)
                z0, z1 = ZCOL0_OF_BATCH[b]
                nc.vector.tensor_reduce(
                    zvals_sb[0:1, b:b + 1], zcols_sb[0:1, z0:z1],
                    axis=AX.X, op=ALU.add)
                nc.gpsimd.dma_start(out_dram[b], acc)

            # ---- PE warmup + pipe priming while x_0 streams in: dummy bf16
            # matmuls open the HAM clock gate, and a garbage e->tanh->scores
            # ->exp chain fills every engine's pipeline so real unit 0 flows
            # through a hot pipe.
            warm_ps = ps_e.tile([f, SC], F32, tag="e", name="warm_ps")
            for w in range(80):
                nc.tensor.matmul(warm_ps[:, 0:128], vrep_sb, vrep_sb,
                                 start=(w == 0), stop=(w == 79))
            e_d = ps_e.tile([f, SC], F32, tag="e", name="e_d")
            for k in range(KD):
                nc.tensor.matmul(e_d[:, 0:128], wet_sb[:, 0, :],
                                 wet_sb[:, k, :], start=(k == 0),
                                 stop=(k == KD - 1))
            h_d = hpool.tile([f, SC], BF16, tag="h", name="h_d")
            nc.scalar.activation(h_d[:, 0:128], e_d[:, 0:128], AF.Tanh,
                                 bias=dvec_sb[:, 0:1])
            sc_d = ps_sc.tile([128, 2, SC], F32, tag="sc", name="sc_d")
            nc.tensor.matmul(sc_d[:, 0, 0:128], vrep_sb, h_d[:, 0:128],
                             start=True, stop=True)
            a_d = apool.tile([128, 2, SC], BF16, tag="a", name="a_d")
            nc.scalar.activation(a_d[:, 0, 0:128], sc_d[:, 0, 0:128],
                                 AF.Exp, bias=vbb_sb,
                                 accum_out=zcols_sb[:, NU:NU + 1])

            # ---- software-pipelined issue over units ----
            for i in range(NU + 4):
                if i + 3 < NU:
                    xts[i + 3] = load_x(i + 3)
                if 0 <= i - 2 < NU:
                    exp_unit(i - 2)
                if 0 <= i - 1 < NU:
                    tanh_unit(i - 1)
                    scores_unit(i - 1)
                if i < NU:
                    e_unit(i)
                if 0 <= i - 2 < NU:
                    wsum_unit(i - 2)
                if 0 <= i - 3 < NU:
                    act_reduces(i - 3)
                    b = UNITS[i - 3][3]
                    if LAST_UNIT_OF_BATCH[b] == i - 3:
                        fin(b)
            nc.gpsimd.dma_start(z_dram, zvals_sb)

    nc.finalize()
    return nc


_NC_CACHE = {}


def _get_nc(key, **kw):
    if key not in _NC_CACHE:
        _NC_CACHE[key] = build_nc(**kw)
    return _NC_CACHE[key]


def make_in_maps(e_hiddens, d_hiddens, We_w, We_b, Wd_w, v_w, v_b, n_cores=N_CORES):
    bp = e_hiddens.shape[0] // n_cores
    bf16 = ml_dtypes.bfloat16

    def arrange(m):  # [D, x] -> [128, KD*x], partition-major tiles
        dd, xx = m.shape
        return np.ascontiguousarray(
            m.reshape(dd // 128, 128, xx).transpose(1, 0, 2).reshape(128, -1))

    wet = arrange(np.ascontiguousarray(We_w.T)).astype(bf16)    # [128, KD*F]
    vrep = np.ascontiguousarray(
        np.repeat(v_w[0][:, None], 128, axis=1)).astype(bf16)   # [F, 128]
    vbb = np.full((128, 1), np.float32(v_b[0]), np.float32)
    maps = []
    for i in range(n_cores):
        xc = e_hiddens[i * bp:(i + 1) * bp]                     # [bp, S, D]
        # xt[pair, p, r*KD*SC + k*SC + s'] = x[b, (2*half+r)*SC+s', k*128+p]
        xt = np.ascontiguousarray(
            xc.reshape(bp, 2, 2, SC, KD, 128).transpose(0, 1, 5, 2, 4, 3)
        ).astype(bf16).reshape(bp * 2, 128, 2 * KD * SC)
        # dvec[f, b] = Wd @ d_hiddens[b] + We_b (the tanh bias), tiny on host
        dvec = (d_hiddens[i * bp:(i + 1) * bp] @ Wd_w.T).T + We_b[:, None]
        maps.append({
            "xt": xt,
            "wet": wet,
            "vrep": vrep,
            "vbb": vbb,
            "dvec": np.ascontiguousarray(dvec, np.float32),
        })
    return maps


def kernel(e_hiddens, d_hiddens, length_mask, We_w, We_b, Wd_w, v_w, v_b,
           _trace=False):
    """Full inputs in, full output out.  length_mask is all-ones (the
    reference adds (1-mask)*1e-32, numerically a no-op)."""
    e_hiddens = np.asarray(e_hiddens, np.float32)
    d_hiddens = np.asarray(d_hiddens, np.float32)
    We_w = np.asarray(We_w, np.float32)
    We_b = np.asarray(We_b, np.float32)
    Wd_w = np.asarray(Wd_w, np.float32)
    v_w = np.asarray(v_w, np.float32)
    v_b = np.asarray(v_b, np.float32)

    nc = _get_nc("full")
    in_maps = make_in_maps(e_hiddens, d_hiddens, We_w, We_b, Wd_w, v_w, v_b)
    res = run_bass_kernel_spmd(nc, in_maps, list(range(N_CORES)), trace=_trace)
    outs = []
    for m in res.results:
        o = m["out"].transpose(0, 2, 1).reshape(BP, D)  # [bp,p,k] -> [bp,d]
        outs.append(o.astype(np.float32) / m["z"].reshape(-1, 1))
    out = np.concatenate(outs, axis=0)
    if _trace:
        kernel.last_results = res
    return out


# revision 36
# speedup vs baseline: 1.0185x; 1.0185x over previous
"""Trainium2 Bass kernel for nn_AttnLayer (additive-attention pooling layer).

Reference computation (per batch b):
    e = e_hiddens @ We_w.T + We_b            # [S, F]
    d = Wd_w @ d_hiddens[b]                  # [F]
    h = tanh(d + e)                          # [S, F]
    s = h @ v_w[0] + v_b                     # [S]
    a = softmax(s)                           # [S]
    out[b] = a @ e_hiddens[b]                # [D]

Strategy (8 cores, data-parallel over batch B=32 -> 4 per core):
  x pre-transposed ON HOST to [d-partition, s-free] bf16, processed in UNITS
  of 1-2 1024-long s-chunks of one batch (one exp + one big DVE mult per
  unit).  Batch 0's first two chunks are single-chunk units so the pipeline
  ramps while the DMA still streams.  Per unit:
    PE : e^T[f,s] = sum_k wet^T @ xt  (bf16); scores via a v-replicated
         stationary -> [128, s] replicated in PSUM
    ACT: h = tanh(e^T + d_b)  ;  a = exp(sc + v_b) over the whole unit in
         one op (accum_out -> unit partial of softmax Z)
    weighted sum x*a:
      DVE: ONE broadcast tensor_tensor (2x bf16 mode) multiplies all 8
           k-slices; slices 0..3 are reduced by an IN-PLACE bf16 halving
           fold tree (also 2x)
      Pool: the final 128->64 fold writes tails into a per-batch tile
      ACT: unit-wide copy+accum reduces for slices 4..7
  Per batch: DVE tensor_reduce folds the partial tails + ACT accum columns
  into [128, 8] per-k sums; host reorders [p,k]->[d] and divides by Z.
  A dummy e->tanh->scores->exp chain plus bf16 warmup matmuls run while x_0
  streams in, opening the PE clock gate and filling every engine pipeline.

Schedule: each engine's FIFO sees ops only after their producers ran one
iteration earlier (exp of unit u-2, tanh/scores of u-1, e of u, weighted sum
of u-2, ACT reduces of u-3), so no head-of-line blocking."""

import numpy as np
import ml_dtypes

import concourse.bass as bass
import concourse.bacc as bacc
import concourse.mybir as mybir
import concourse.tile as tile
from concourse.bass_utils import run_bass_kernel_spmd

F32 = mybir.dt.float32
BF16 = mybir.dt.bfloat16
AF = mybir.ActivationFunctionType
ALU = mybir.AluOpType
AX = mybir.AxisListType

N_CORES = 8
B, S, D, F = 32, 4096, 1024, 128
BP = B // N_CORES          # batches per core
KD = D // 128              # d-slices (partition groups)
SC = 1024                  # s-chunk
NCH = S // SC              # chunks per batch
NP = BP * (NCH // 2)       # record-pairs in DRAM layout

NKF = 4                    # k-slices reduced by the DVE fold tree
TW = 64                    # fold-tail width

# units: (dram_pair, r0, nr, batch, c0, pta_col).  Batch 0's first two
# chunks are single-record units to shorten the pipeline ramp.
UNITS = [(0, 0, 1, 0, 0, 0), (0, 1, 1, 0, 1, 1), (1, 0, 2, 0, 2, 2)] + [
    (p, 0, 2, p // 2, 2 * (p % 2), p % 2) for p in range(2, NP)]
NU = len(UNITS)
LAST_UNIT_OF_BATCH = {u[3]: i for i, u in enumerate(UNITS)}
ZCOL0_OF_BATCH = {0: (0, 3)}
for _i, _u in enumerate(UNITS):
    if _u[3] > 0:
        lo, hi = ZCOL0_OF_BATCH.get(_u[3], (_i, _i))
        ZCOL0_OF_BATCH[_u[3]] = (min(lo, _i), _i + 1)


def build_nc(bp=BP, s=S, d=D, f=F):
    nc = bacc.Bacc("TRN2", target_bir_lowering=False, debug=False)

    xt_dram = nc.dram_tensor("xt", [NP, 128, 2 * KD * SC], BF16,
                             kind="ExternalInput").ap()
    wet_dram = nc.dram_tensor("wet", [128, KD * f], BF16, kind="ExternalInput").ap()
    vrep_dram = nc.dram_tensor("vrep", [f, 128], BF16, kind="ExternalInput").ap()
    vbb_dram = nc.dram_tensor("vbb", [128, 1], F32, kind="ExternalInput").ap()
    dvec_dram = nc.dram_tensor("dvec", [f, bp], F32, kind="ExternalInput").ap()
    pt_dram = nc.dram_tensor("pt", [bp, 128, NKF * NCH * TW], BF16,
                             kind="ExternalOutput").ap()
    pta_dram = nc.dram_tensor("pta", [bp, 128, (KD - NKF) * 3], F32,
                              kind="ExternalOutput").ap()
    zc_dram = nc.dram_tensor("zc", [1, NU + 1], F32, kind="ExternalOutput").ap()

    with tile.TileContext(nc) as tc:
        with (
            tc.tile_pool(name="const", bufs=1) as const,
            tc.tile_pool(name="xpool", bufs=3) as xpool,
            tc.tile_pool(name="mpool", bufs=2) as mpool,
            tc.tile_pool(name="hpool", bufs=3) as hpool,
            tc.tile_pool(name="apool", bufs=2) as apool,
            tc.tile_pool(name="ppool", bufs=2) as ppool,
            tc.tile_pool(name="ptpool", bufs=2) as ptpool,
            tc.tile_pool(name="opool", bufs=2) as opool,
            tc.tile_pool(name="ps_e", bufs=2, space="PSUM") as ps_e,
            tc.tile_pool(name="ps_sc", bufs=1, space="PSUM") as ps_sc,
        ):
            # ---- constants on parallel DMA queues (x-loads use sync) ----
            vrep_sb = const.tile([f, 128], BF16)
            nc.scalar.dma_start(vrep_sb, vrep_dram)
            wet_sb = const.tile([128, KD, f], BF16)
            nc.scalar.dma_start(wet_sb, wet_dram.rearrange("p (k f) -> p k f", k=KD))
            vbb_sb = const.tile([128, 1], F32)
            nc.gpsimd.dma_start(vbb_sb, vbb_dram)
            dvec_sb = const.tile([f, bp], F32)
            nc.gpsimd.dma_start(dvec_sb, dvec_dram)
            zcols_sb = const.tile([128, NU + 1], F32)

            def load_x(u):
                pair, r0, nr, _, _, _ = UNITS[u]
                xt = xpool.tile([128, 2, KD, SC], BF16, tag="x", name=f"x_{u}")
                src = xt_dram[pair].rearrange("p (r k s) -> p r k s", r=2, k=KD)
                for rr in range(nr):
                    nc.sync.dma_start(xt[:, rr], src[:, r0 + rr])
                return xt

            xts = {u: load_x(u) for u in range(min(3, NU))}

            state = {}
            pts = {}
            ptas = {}

            def get_pt(b):
                if b not in pts:
                    pts[b] = ptpool.tile([128, NKF, NCH, TW], BF16, tag="pt",
                                         name=f"pt_{b}")
                    ptas[b] = ptpool.tile([128, KD - NKF, 3], F32, tag="pta",
                                          name=f"pta_{b}")
                return pts[b], ptas[b]

            def e_unit(u):
                nr = UNITS[u][2]
                xt = xts[u]
                es = []
                for rr in range(nr):
                    e_ps = ps_e.tile([f, SC], F32, tag="e", name=f"e_{u}_{rr}")
                    for k in range(KD):
                        for h2 in range(2):
                            sl = slice(h2 * 512, (h2 + 1) * 512)
                            nc.tensor.matmul(
                                e_ps[:, sl], wet_sb[:, k, :], xt[:, rr, k, sl],
                                start=(k == 0), stop=(k == KD - 1))
                    es.append(e_ps)
                state[u] = {"xt": xt, "e": es}

            def tanh_unit(u):
                b = UNITS[u][3]
                es = state[u].pop("e")
                hs = []
                for rr, e_ps in enumerate(es):
                    h_sb = hpool.tile([f, SC], BF16, tag="h", name=f"h_{u}_{rr}")
                    nc.scalar.activation(h_sb, e_ps, AF.Tanh,
                                         bias=dvec_sb[:, b:b + 1])
                    hs.append(h_sb)
                state[u]["h"] = hs

            def scores_unit(u):
                hs = state[u].pop("h")
                sc2 = ps_sc.tile([128, 2, SC], F32, tag="sc", name=f"sc_{u}")
                for rr, h_sb in enumerate(hs):
                    for h2 in range(2):
                        sl = slice(h2 * 512, (h2 + 1) * 512)
                        nc.tensor.matmul(sc2[:, rr, sl], vrep_sb, h_sb[:, sl],
                                         start=True, stop=True)
                state[u]["sc2"] = sc2

            def exp_unit(u):
                nr = UNITS[u][2]
                sc2 = state[u].pop("sc2")
                a2 = apool.tile([128, 2, SC], BF16, tag="a", name=f"a_{u}")
                nc.scalar.activation(
                    a2[:, 0:nr].rearrange("p r s -> p (r s)"),
                    sc2[:, 0:nr].rearrange("p r s -> p (r s)"),
                    AF.Exp, bias=vbb_sb, accum_out=zcols_sb[:, u:u + 1])
                state[u]["a2"] = a2

            def wsum_unit(u):
                _, _, nr, b, c0, _ = UNITS[u]
                st = state[u]
                xt, a2 = st.pop("xt"), st.pop("a2")
                xts.pop(u)
                pt, _ = get_pt(b)
                # DVE: one broadcast mult for all 8 k-slices of the unit
                # (last unit: ACT-reduced slices first so the drain overlaps)
                m2 = mpool.tile([128, 2, KD, SC], BF16, tag="m", name=f"m_{u}")
                ksls = ([slice(NKF, KD), slice(0, NKF)] if u == NU - 1
                        else [slice(0, KD)])
                for ksl in ksls:
                    nc.vector.tensor_tensor(
                        m2[:, 0:nr, ksl, :], xt[:, 0:nr, ksl, :],
                        a2[:, 0:nr].unsqueeze(2).broadcast_to(
                            [128, nr, ksl.stop - ksl.start, SC]),
                        op=ALU.mult)
                # DVE: in-place fold tree on slices 0..NKF-1 down to 2*TW
                h = SC // 2
                while h >= 2 * TW:
                    nc.vector.tensor_tensor(
                        m2[:, 0:nr, 0:NKF, 0:h], m2[:, 0:nr, 0:NKF, 0:h],
                        m2[:, 0:nr, 0:NKF, h:2 * h], op=ALU.add)
                    h //= 2
                # last fold drops tails into the per-batch partial tile
                nc.vector.tensor_tensor(
                    pt[:, :, c0:c0 + nr, :],
                    m2[:, 0:nr, 0:NKF, 0:TW].rearrange("p r k s -> p k r s"),
                    m2[:, 0:nr, 0:NKF, TW:2 * TW].rearrange("p r k s -> p k r s"),
                    op=ALU.add)
                st["m2"] = m2

            def act_reduces(u):
                _, _, nr, b, _, pcol = UNITS[u]
                m2 = state.pop(u)["m2"]
                _, pta = get_pt(b)
                # ACT: unit-wide copy+accum reduces for k=NKF..7 (chunks of
                # one batch summed together; fin() sums chunks anyway)
                da = ppool.tile([128, 2, SC], BF16, tag="da", name=f"da_{u}")
                for ki in range(NKF, KD):
                    nc.scalar.activation(
                        da[:, 0:nr], m2[:, 0:nr, ki, :], AF.Copy,
                        accum_out=pta[:, ki - NKF, pcol:pcol + 1])

            def fin(b):
                # raw partials go to the host; it does the tiny final sums
                pt, pta = pts.pop(b), ptas.pop(b)
                nc.gpsimd.dma_start(pt_dram[b],
                                    pt.rearrange("p k c s -> p (k c s)"))
                nc.gpsimd.dma_start(pta_dram[b],
                                    pta.rearrange("p k c -> p (k c)"))

            # ---- PE warmup + pipe priming while x_0 streams in: dummy bf16
            # matmuls open the HAM clock gate, and a garbage e->tanh->scores
            # ->exp chain fills every engine's pipeline so real unit 0 flows
            # through a hot pipe.
            warm_ps = ps_e.tile([f, SC], F32, tag="e", name="warm_ps")
            for w in range(40):
                nc.tensor.matmul(warm_ps[:, 0:128], vrep_sb, vrep_sb,
                                 start=(w == 0), stop=(w == 39))
            e_d = ps_e.tile([f, SC], F32, tag="e", name="e_d")
            for k in range(KD):
                nc.tensor.matmul(e_d[:, 0:128], wet_sb[:, 0, :],
                                 wet_sb[:, k, :], start=(k == 0),
                                 stop=(k == KD - 1))
            h_d = hpool.tile([f, SC], BF16, tag="h", name="h_d")
            nc.scalar.activation(h_d[:, 0:128], e_d[:, 0:128], AF.Tanh,
                                 bias=dvec_sb[:, 0:1])
            sc_d = ps_sc.tile([128, 2, SC], F32, tag="sc", name="sc_d")
            nc.tensor.matmul(sc_d[:, 0, 0:128], vrep_sb, h_d[:, 0:128],
                             start=True, stop=True)
            a_d = apool.tile([128, 2, SC], BF16, tag="a", name="a_d")
            nc.scalar.activation(a_d[:, 0, 0:128], sc_d[:, 0, 0:128],
                                 AF.Exp, bias=vbb_sb,
                                 accum_out=zcols_sb[:, NU:NU + 1])

            # ---- software-pipelined issue over units ----
            for i in range(NU + 4):
                if i + 3 < NU:
                    xts[i + 3] = load_x(i + 3)
                if 0 <= i - 2 < NU:
                    exp_unit(i - 2)
                if 0 <= i - 1 < NU:
                    tanh_unit(i - 1)
                    scores_unit(i - 1)
                if i < NU:
                    e_unit(i)
                if 0 <= i - 2 < NU:
                    wsum_unit(i - 2)
                if 0 <= i - 3 < NU:
                    act_reduces(i - 3)
                    b = UNITS[i - 3][3]
                    if LAST_UNIT_OF_BATCH[b] == i - 3:
                        fin(b)
            nc.gpsimd.dma_start(zc_dram, zcols_sb[0:1, :])

    nc.finalize()
    return nc


_NC_CACHE = {}


def _get_nc(key, **kw):
    if key not in _NC_CACHE:
        _NC_CACHE[key] = build_nc(**kw)
    return _NC_CACHE[key]


def make_in_maps(e_hiddens, d_hiddens, We_w, We_b, Wd_w, v_w, v_b, n_cores=N_CORES):
    bp = e_hiddens.shape[0] // n_cores
    bf16 = ml_dtypes.bfloat16

    def arrange(m):  # [D, x] -> [128, KD*x], partition-major tiles
        dd, xx = m.shape
        return np.ascontiguousarray(
            m.reshape(dd // 128, 128, xx).transpose(1, 0, 2).reshape(128, -1))

    wet = arrange(np.ascontiguousarray(We_w.T)).astype(bf16)    # [128, KD*F]
    vrep = np.ascontiguousarray(
        np.repeat(v_w[0][:, None], 128, axis=1)).astype(bf16)   # [F, 128]
    vbb = np.full((128, 1), np.float32(v_b[0]), np.float32)
    maps = []
    for i in range(n_cores):
        xc = e_hiddens[i * bp:(i + 1) * bp]                     # [bp, S, D]
        # xt[pair, p, r*KD*SC + k*SC + s'] = x[b, (2*half+r)*SC+s', k*128+p]
        xt = np.ascontiguousarray(
            xc.reshape(bp, 2, 2, SC, KD, 128).transpose(0, 1, 5, 2, 4, 3)
        ).astype(bf16).reshape(bp * 2, 128, 2 * KD * SC)
        # dvec[f, b] = Wd @ d_hiddens[b] + We_b (the tanh bias), tiny on host
        dvec = (d_hiddens[i * bp:(i + 1) * bp] @ Wd_w.T).T + We_b[:, None]
        maps.append({
            "xt": xt,
            "wet": wet,
            "vrep": vrep,
            "vbb": vbb,
            "dvec": np.ascontiguousarray(dvec, np.float32),
        })
    return maps


def kernel(e_hiddens, d_hiddens, length_mask, We_w, We_b, Wd_w, v_w, v_b,
           _trace=False):
    """Full inputs in, full output out.  length_mask is all-ones (the
    reference adds (1-mask)*1e-32, numerically a no-op)."""
    e_hiddens = np.asarray(e_hiddens, np.float32)
    d_hiddens = np.asarray(d_hiddens, np.float32)
    We_w = np.asarray(We_w, np.float32)
    We_b = np.asarray(We_b, np.float32)
    Wd_w = np.asarray(Wd_w, np.float32)
    v_w = np.asarray(v_w, np.float32)
    v_b = np.asarray(v_b, np.float32)

    nc = _get_nc("full")
    in_maps = make_in_maps(e_hiddens, d_hiddens, We_w, We_b, Wd_w, v_w, v_b)
    res = run_bass_kernel_spmd(nc, in_maps, list(range(N_CORES)), trace=_trace)
    outs = []
    for m in res.results:
        # pt[b, p, k, c, tw] bf16 fold tails; pta[b, p, k', col] f32
        pt = m["pt"].astype(np.float32).reshape(BP, 128, NKF, NCH, TW)
        pta = m["pta"].reshape(BP, 128, KD - NKF, 3)
        zc = m["zc"].reshape(-1)
        acc = np.empty((BP, 128, KD), np.float32)
        acc[:, :, 0:NKF] = pt.sum(axis=(3, 4))
        z = np.empty((BP, 1), np.float32)
        for b in range(BP):
            ncols = 3 if b == 0 else 2
            acc[b, :, NKF:KD] = pta[b, :, :, 0:ncols].sum(axis=2)
            z0, z1 = ZCOL0_OF_BATCH[b]
            z[b, 0] = zc[z0:z1].sum()
        o = acc.transpose(0, 2, 1).reshape(BP, D)  # [bp,p,k] -> [bp,d]
        outs.append(o / z)
    out = np.concatenate(outs, axis=0)
    if _trace:
        kernel.last_results = res
    return out


# revision 39
# speedup vs baseline: 1.0207x; 1.0022x over previous
"""Trainium2 Bass kernel for nn_AttnLayer (additive-attention pooling layer).

Reference computation (per batch b):
    e = e_hiddens @ We_w.T + We_b            # [S, F]
    d = Wd_w @ d_hiddens[b]                  # [F]
    h = tanh(d + e)                          # [S, F]
    s = h @ v_w[0] + v_b                     # [S]
    a = softmax(s)                           # [S]
    out[b] = a @ e_hiddens[b]                # [D]

Strategy (8 cores, data-parallel over batch B=32 -> 4 per core):
  x pre-transposed ON HOST to [d-partition, s-free] bf16, processed in UNITS
  of 1-2 1024-long s-chunks of one batch (one exp + one big DVE mult per
  unit).  Batch 0's first two chunks are single-chunk units so the pipeline
  ramps while the DMA still streams.  Per unit:
    PE : e^T[f,s] = sum_k wet^T @ xt  (bf16); scores via a v-replicated
         stationary -> [128, s] replicated in PSUM
    ACT: h = tanh(e^T + d_b)  ;  a = exp(sc + v_b) over the whole unit in
         one op (accum_out -> unit partial of softmax Z)
    weighted sum x*a:
      DVE: ONE broadcast tensor_tensor (2x bf16 mode) multiplies all 8
           k-slices; slices 0..3 are reduced by an IN-PLACE bf16 halving
           fold tree (also 2x)
      Pool: the final 128->64 fold writes tails into a per-batch tile
      ACT: unit-wide copy+accum reduces for slices 4..7
  Per batch: DVE tensor_reduce folds the partial tails + ACT accum columns
  into [128, 8] per-k sums; host reorders [p,k]->[d] and divides by Z.
  A dummy e->tanh->scores->exp chain plus bf16 warmup matmuls run while x_0
  streams in, opening the PE clock gate and filling every engine pipeline.

Schedule: each engine's FIFO sees ops only after their producers ran one
iteration earlier (exp of unit u-2, tanh/scores of u-1, e of u, weighted sum
of u-2, ACT reduces of u-3), so no head-of-line blocking."""

import numpy as np
import ml_dtypes

import concourse.bass as bass
import concourse.bacc as bacc
import concourse.mybir as mybir
import concourse.tile as tile
from concourse.bass_utils import run_bass_kernel_spmd

F32 = mybir.dt.float32
BF16 = mybir.dt.bfloat16
AF = mybir.ActivationFunctionType
ALU = mybir.AluOpType
AX = mybir.AxisListType

N_CORES = 8
B, S, D, F = 32, 4096, 1024, 128
BP = B // N_CORES          # batches per core
KD = D // 128              # d-slices (partition groups)
SC = 1024                  # s-chunk
NCH = S // SC              # chunks per batch
NP = BP * (NCH // 2)       # record-pairs in DRAM layout

NKF = 4                    # k-slices reduced by the DVE fold tree
TW = 64                    # fold-tail width

# units: (dram_pair, r0, nr, batch, c0, pta_col)
UNITS = [(p, 0, 2, p // 2, 2 * (p % 2), p % 2) for p in range(NP)]
NU = len(UNITS)
LAST_UNIT_OF_BATCH = {u[3]: i for i, u in enumerate(UNITS)}
ZCOL0_OF_BATCH = {}
for _i, _u in enumerate(UNITS):
    lo, hi = ZCOL0_OF_BATCH.get(_u[3], (_i, _i))
    ZCOL0_OF_BATCH[_u[3]] = (min(lo, _i), _i + 1)


def build_nc(bp=BP, s=S, d=D, f=F):
    nc = bacc.Bacc("TRN2", target_bir_lowering=False, debug=False)

    xt_dram = nc.dram_tensor("xt", [NP, 128, 2 * KD * SC], BF16,
                             kind="ExternalInput").ap()
    wet_dram = nc.dram_tensor("wet", [128, KD * f], BF16, kind="ExternalInput").ap()
    vrep_dram = nc.dram_tensor("vrep", [f, 128], BF16, kind="ExternalInput").ap()
    vbb_dram = nc.dram_tensor("vbb", [128, 1], F32, kind="ExternalInput").ap()
    dvec_dram = nc.dram_tensor("dvec", [f, bp], F32, kind="ExternalInput").ap()
    pt_dram = nc.dram_tensor("pt", [bp, 128, NKF * NCH * TW], BF16,
                             kind="ExternalOutput").ap()
    pta_dram = nc.dram_tensor("pta", [bp, 128, (KD - NKF) * 3], F32,
                              kind="ExternalOutput").ap()
    zc_dram = nc.dram_tensor("zc", [1, NU + 1], F32, kind="ExternalOutput").ap()

    with tile.TileContext(nc) as tc:
        with (
            tc.tile_pool(name="const", bufs=1) as const,
            tc.tile_pool(name="xpool", bufs=3) as xpool,
            tc.tile_pool(name="mpool", bufs=2) as mpool,
            tc.tile_pool(name="hpool", bufs=3) as hpool,
            tc.tile_pool(name="apool", bufs=2) as apool,
            tc.tile_pool(name="ppool", bufs=2) as ppool,
            tc.tile_pool(name="ptpool", bufs=2) as ptpool,
            tc.tile_pool(name="opool", bufs=2) as opool,
            tc.tile_pool(name="ps_e", bufs=2, space="PSUM") as ps_e,
            tc.tile_pool(name="ps_sc", bufs=1, space="PSUM") as ps_sc,
        ):
            # ---- constants on parallel DMA queues (x-loads use sync) ----
            vrep_sb = const.tile([f, 128], BF16)
            nc.scalar.dma_start(vrep_sb, vrep_dram)
            wet_sb = const.tile([128, KD, f], BF16)
            nc.scalar.dma_start(wet_sb, wet_dram.rearrange("p (k f) -> p k f", k=KD))
            vbb_sb = const.tile([128, 1], F32)
            nc.gpsimd.dma_start(vbb_sb, vbb_dram)
            dvec_sb = const.tile([f, bp], F32)
            nc.gpsimd.dma_start(dvec_sb, dvec_dram)
            zcols_sb = const.tile([128, NU + 1], F32)

            def load_x(u):
                pair, r0, nr, _, _, _ = UNITS[u]
                xt = xpool.tile([128, 2, KD, SC], BF16, tag="x", name=f"x_{u}")
                src = xt_dram[pair].rearrange("p (r k s) -> p r k s", r=2, k=KD)
                for rr in range(nr):
                    nc.sync.dma_start(xt[:, rr], src[:, r0 + rr])
                return xt

            xts = {u: load_x(u) for u in range(min(3, NU))}

            state = {}
            pts = {}
            ptas = {}

            def get_pt(b):
                if b not in pts:
                    pts[b] = ptpool.tile([128, NKF, NCH, TW], BF16, tag="pt",
                                         name=f"pt_{b}")
                    ptas[b] = ptpool.tile([128, KD - NKF, 3], F32, tag="pta",
                                          name=f"pta_{b}")
                return pts[b], ptas[b]

            def e_unit(u):
                nr = UNITS[u][2]
                xt = xts[u]
                es = []
                for rr in range(nr):
                    e_ps = ps_e.tile([f, SC], F32, tag="e", name=f"e_{u}_{rr}")
                    for k in range(KD):
                        for h2 in range(2):
                            sl = slice(h2 * 512, (h2 + 1) * 512)
                            nc.tensor.matmul(
                                e_ps[:, sl], wet_sb[:, k, :], xt[:, rr, k, sl],
                                start=(k == 0), stop=(k == KD - 1))
                    es.append(e_ps)
                state[u] = {"xt": xt, "e": es}

            def tanh_unit(u):
                b = UNITS[u][3]
                es = state[u].pop("e")
                hs = []
                for rr, e_ps in enumerate(es):
                    h_sb = hpool.tile([f, SC], BF16, tag="h", name=f"h_{u}_{rr}")
                    nc.scalar.activation(h_sb, e_ps, AF.Tanh,
                                         bias=dvec_sb[:, b:b + 1])
                    hs.append(h_sb)
                state[u]["h"] = hs

            def scores_unit(u):
                hs = state[u].pop("h")
                sc2 = ps_sc.tile([128, 2, SC], F32, tag="sc", name=f"sc_{u}")
                for rr, h_sb in enumerate(hs):
                    for h2 in range(2):
                        sl = slice(h2 * 512, (h2 + 1) * 512)
                        nc.tensor.matmul(sc2[:, rr, sl], vrep_sb, h_sb[:, sl],
                                         start=True, stop=True)
                state[u]["sc2"] = sc2

            def exp_unit(u):
                nr = UNITS[u][2]
                sc2 = state[u].pop("sc2")
                a2 = apool.tile([128, 2, SC], BF16, tag="a", name=f"a_{u}")
                nc.scalar.activation(
                    a2[:, 0:nr].rearrange("p r s -> p (r s)"),
                    sc2[:, 0:nr].rearrange("p r s -> p (r s)"),
                    AF.Exp, bias=vbb_sb, accum_out=zcols_sb[:, u:u + 1])
                state[u]["a2"] = a2

            def wsum_unit(u):
                _, _, nr, b, c0, _ = UNITS[u]
                st = state[u]
                xt, a2 = st.pop("xt"), st.pop("a2")
                xts.pop(u)
                pt, _ = get_pt(b)
                # DVE: one broadcast mult for all 8 k-slices of the unit
                # (last unit: ACT-reduced slices first so the drain overlaps)
                m2 = mpool.tile([128, 2, KD, SC], BF16, tag="m", name=f"m_{u}")
                ksls = ([slice(NKF, KD), slice(0, NKF)] if u == NU - 1
                        else [slice(0, KD)])
                for ksl in ksls:
                    nc.vector.tensor_tensor(
                        m2[:, 0:nr, ksl, :], xt[:, 0:nr, ksl, :],
                        a2[:, 0:nr].unsqueeze(2).broadcast_to(
                            [128, nr, ksl.stop - ksl.start, SC]),
                        op=ALU.mult)
                # DVE: in-place fold tree on slices 0..NKF-1 down to 2*TW
                h = SC // 2
                while h >= 2 * TW:
                    nc.vector.tensor_tensor(
                        m2[:, 0:nr, 0:NKF, 0:h], m2[:, 0:nr, 0:NKF, 0:h],
                        m2[:, 0:nr, 0:NKF, h:2 * h], op=ALU.add)
                    h //= 2
                # last fold drops tails into the per-batch partial tile
                nc.vector.tensor_tensor(
                    pt[:, :, c0:c0 + nr, :],
                    m2[:, 0:nr, 0:NKF, 0:TW].rearrange("p r k s -> p k r s"),
                    m2[:, 0:nr, 0:NKF, TW:2 * TW].rearrange("p r k s -> p k r s"),
                    op=ALU.add)
                st["m2"] = m2

            def act_reduces(u):
                _, _, nr, b, _, pcol = UNITS[u]
                m2 = state.pop(u)["m2"]
                _, pta = get_pt(b)
                # ACT: unit-wide copy+accum reduces for k=NKF..7 (chunks of
                # one batch summed together; fin() sums chunks anyway)
                da = ppool.tile([128, 2, SC], BF16, tag="da", name=f"da_{u}")
                for ki in range(NKF, KD):
                    nc.scalar.activation(
                        da[:, 0:nr], m2[:, 0:nr, ki, :], AF.Copy,
                        accum_out=pta[:, ki - NKF, pcol:pcol + 1])

            def fin(b):
                # raw partials go to the host; it does the tiny final sums
                pt, pta = pts.pop(b), ptas.pop(b)
                nc.gpsimd.dma_start(pt_dram[b],
                                    pt.rearrange("p k c s -> p (k c s)"))
                nc.gpsimd.dma_start(pta_dram[b],
                                    pta.rearrange("p k c -> p (k c)"))

            # ---- PE warmup + pipe priming while x_0 streams in: dummy bf16
            # matmuls open the HAM clock gate, and a garbage e->tanh->scores
            # ->exp chain fills every engine's pipeline so real unit 0 flows
            # through a hot pipe.
            warm_ps = ps_e.tile([f, SC], F32, tag="e", name="warm_ps")
            for w in range(40):
                nc.tensor.matmul(warm_ps[:, 0:128], vrep_sb, vrep_sb,
                                 start=(w == 0), stop=(w == 39))
            e_d = ps_e.tile([f, SC], F32, tag="e", name="e_d")
            for k in range(KD):
                nc.tensor.matmul(e_d[:, 0:128], wet_sb[:, 0, :],
                                 wet_sb[:, k, :], start=(k == 0),
                                 stop=(k == KD - 1))
            h_d = hpool.tile([f, SC], BF16, tag="h", name="h_d")
            nc.scalar.activation(h_d[:, 0:128], e_d[:, 0:128], AF.Tanh,
                                 bias=dvec_sb[:, 0:1])
            sc_d = ps_sc.tile([128, 2, SC], F32, tag="sc", name="sc_d")
            nc.tensor.matmul(sc_d[:, 0, 0:128], vrep_sb, h_d[:, 0:128],
                             start=True, stop=True)
            a_d = apool.tile([128, 2, SC], BF16, tag="a", name="a_d")
            nc.scalar.activation(a_d[:, 0, 0:128], sc_d[:, 0, 0:128],
                                 AF.Exp, bias=vbb_sb,
                                 accum_out=zcols_sb[:, NU:NU + 1])

            # ---- software-pipelined issue over units ----
            for i in range(NU + 4):
                if i + 3 < NU:
                    xts[i + 3] = load_x(i + 3)
                if 0 <= i - 2 < NU:
                    exp_unit(i - 2)
                if 0 <= i - 1 < NU:
                    tanh_unit(i - 1)
                    scores_unit(i - 1)
                if i < NU:
                    e_unit(i)
                if 0 <= i - 2 < NU:
                    wsum_unit(i - 2)
                if 0 <= i - 3 < NU:
                    act_reduces(i - 3)
                    b = UNITS[i - 3][3]
                    if LAST_UNIT_OF_BATCH[b] == i - 3:
                        fin(b)
            nc.gpsimd.dma_start(zc_dram, zcols_sb[0:1, :])

    nc.finalize()
    return nc


_NC_CACHE = {}


def _get_nc(key, **kw):
    if key not in _NC_CACHE:
        _NC_CACHE[key] = build_nc(**kw)
    return _NC_CACHE[key]


def make_in_maps(e_hiddens, d_hiddens, We_w, We_b, Wd_w, v_w, v_b, n_cores=N_CORES):
    bp = e_hiddens.shape[0] // n_cores
    bf16 = ml_dtypes.bfloat16

    def arrange(m):  # [D, x] -> [128, KD*x], partition-major tiles
        dd, xx = m.shape
        return np.ascontiguousarray(
            m.reshape(dd // 128, 128, xx).transpose(1, 0, 2).reshape(128, -1))

    wet = arrange(np.ascontiguousarray(We_w.T)).astype(bf16)    # [128, KD*F]
    vrep = np.ascontiguousarray(
        np.repeat(v_w[0][:, None], 128, axis=1)).astype(bf16)   # [F, 128]
    vbb = np.full((128, 1), np.float32(v_b[0]), np.float32)
    maps = []
    for i in range(n_cores):
        xc = e_hiddens[i * bp:(i + 1) * bp]                     # [bp, S, D]
        # xt[pair, p, r*KD*SC + k*SC + s'] = x[b, (2*half+r)*SC+s', k*128+p]
        xt = np.ascontiguousarray(
            xc.reshape(bp, 2, 2, SC, KD, 128).transpose(0, 1, 5, 2, 4, 3)
        ).astype(bf16).reshape(bp * 2, 128, 2 * KD * SC)
        # dvec[f, b] = Wd @ d_hiddens[b] + We_b (the tanh bias), tiny on host
        dvec = (d_hiddens[i * bp:(i + 1) * bp] @ Wd_w.T).T + We_b[:, None]
        maps.append({
            "xt": xt,
            "wet": wet,
            "vrep": vrep,
            "vbb": vbb,
            "dvec": np.ascontiguousarray(dvec, np.float32),
        })
    return maps


def kernel(e_hiddens, d_hiddens, length_mask, We_w, We_b, Wd_w, v_w, v_b,
           _trace=False):
    """Full inputs in, full output out.  length_mask is all-ones (the
    reference adds (1-mask)*1e-32, numerically a no-op)."""
    e_hiddens = np.asarray(e_hiddens, np.float32)
    d_hiddens = np.asarray(d_hiddens, np.float32)
    We_w = np.asarray(We_w, np.float32)
    We_b = np.asarray(We_b, np.float32)
    Wd_w = np.asarray(Wd_w, np.float32)
    v_w = np.asarray(v_w, np.float32)
    v_b = np.asarray(v_b, np.float32)

    nc = _get_nc("full")
    in_maps = make_in_maps(e_hiddens, d_hiddens, We_w, We_b, Wd_w, v_w, v_b)
    res = run_bass_kernel_spmd(nc, in_maps, list(range(N_CORES)), trace=_trace)
    outs = []
    for m in res.results:
        # pt[b, p, k, c, tw] bf16 fold tails; pta[b, p, k', col] f32
        pt = m["pt"].astype(np.float32).reshape(BP, 128, NKF, NCH, TW)
        pta = m["pta"].reshape(BP, 128, KD - NKF, 3)
        zc = m["zc"].reshape(-1)
        acc = np.empty((BP, 128, KD), np.float32)
        acc[:, :, 0:NKF] = pt.sum(axis=(3, 4))
        z = np.empty((BP, 1), np.float32)
        for b in range(BP):
            ncols = max(u[5] for u in UNITS if u[3] == b) + 1
            acc[b, :, NKF:KD] = pta[b, :, :, 0:ncols].sum(axis=2)
            z0, z1 = ZCOL0_OF_BATCH[b]
            z[b, 0] = zc[z0:z1].sum()
        o = acc.transpose(0, 2, 1).reshape(BP, D)  # [bp,p,k] -> [bp,d]
        outs.append(o / z)
    out = np.concatenate(outs, axis=0)
    if _trace:
        kernel.last_results = res
    return out


# revision 40
# speedup vs baseline: 1.0780x; 1.0561x over previous
"""Trainium2 Bass kernel for nn_AttnLayer (additive-attention pooling layer).

Reference computation (per batch b):
    e = e_hiddens @ We_w.T + We_b            # [S, F]
    d = Wd_w @ d_hiddens[b]                  # [F]
    h = tanh(d + e)                          # [S, F]
    s = h @ v_w[0] + v_b                     # [S]
    a = softmax(s)                           # [S]
    out[b] = a @ e_hiddens[b]                # [D]

Strategy (8 cores, data-parallel over batch B=32 -> 4 per core):
  x pre-transposed ON HOST to [d-partition, s-free] bf16, processed in UNITS
  of 1-2 1024-long s-chunks of one batch (one exp + one big DVE mult per
  unit).  Batch 0's first two chunks are single-chunk units so the pipeline
  ramps while the DMA still streams.  Per unit:
    PE : e^T[f,s] = sum_k wet^T @ xt  (bf16); scores via a v-replicated
         stationary -> [128, s] replicated in PSUM
    ACT: h = tanh(e^T + d_b)  ;  a = exp(sc + v_b) over the whole unit in
         one op (accum_out -> unit partial of softmax Z)
    weighted sum x*a:
      DVE: ONE broadcast tensor_tensor (2x bf16 mode) multiplies all 8
           k-slices; slices 0..3 are reduced by an IN-PLACE bf16 halving
           fold tree (also 2x)
      Pool: the final 128->64 fold writes tails into a per-batch tile
      ACT: unit-wide copy+accum reduces for slices 4..7
  Per batch: DVE tensor_reduce folds the partial tails + ACT accum columns
  into [128, 8] per-k sums; host reorders [p,k]->[d] and divides by Z.
  A dummy e->tanh->scores->exp chain plus bf16 warmup matmuls run while x_0
  streams in, opening the PE clock gate and filling every engine pipeline.

Schedule: each engine's FIFO sees ops only after their producers ran one
iteration earlier (exp of unit u-2, tanh/scores of u-1, e of u, weighted sum
of u-2, ACT reduces of u-3), so no head-of-line blocking."""

import numpy as np
import ml_dtypes

import concourse.bass as bass
import concourse.bacc as bacc
import concourse.mybir as mybir
import concourse.tile as tile
from concourse.bass_utils import run_bass_kernel_spmd

F32 = mybir.dt.float32
BF16 = mybir.dt.bfloat16
AF = mybir.ActivationFunctionType
ALU = mybir.AluOpType
AX = mybir.AxisListType

N_CORES = 8
B, S, D, F = 32, 4096, 1024, 128
BP = B // N_CORES          # batches per core
KD = D // 128              # d-slices (partition groups)
SC = 1024                  # s-chunk
NCH = S // SC              # chunks per batch
NP = BP * (NCH // 2)       # record-pairs in DRAM layout

NKF = 4                    # k-slices reduced by the DVE fold tree
TW = 64                    # fold-tail width

# units: (dram_pair, r0, nr, batch, c0, pta_col)
UNITS = [(p, 0, 2, p // 2, 2 * (p % 2), p % 2) for p in range(NP)]
NU = len(UNITS)
LAST_UNIT_OF_BATCH = {u[3]: i for i, u in enumerate(UNITS)}
ZCOL0_OF_BATCH = {}
for _i, _u in enumerate(UNITS):
    lo, hi = ZCOL0_OF_BATCH.get(_u[3], (_i, _i))
    ZCOL0_OF_BATCH[_u[3]] = (min(lo, _i), _i + 1)


def build_nc(bp=BP, s=S, d=D, f=F):
    nc = bacc.Bacc("TRN2", target_bir_lowering=False, debug=False)

    xt_dram = nc.dram_tensor("xt", [NP, 128, 2 * KD * SC], BF16,
                             kind="ExternalInput").ap()
    wet_dram = nc.dram_tensor("wet", [128, KD * f], BF16, kind="ExternalInput").ap()
    vrep_dram = nc.dram_tensor("vrep", [f, 128], BF16, kind="ExternalInput").ap()
    vbb_dram = nc.dram_tensor("vbb", [128, 1], F32, kind="ExternalInput").ap()
    dvec_dram = nc.dram_tensor("dvec", [f, bp], F32, kind="ExternalInput").ap()
    pt_dram = nc.dram_tensor("pt", [bp, 128, NKF * NCH * TW], BF16,
                             kind="ExternalOutput").ap()
    pta_dram = nc.dram_tensor("pta", [bp, 128, (KD - NKF) * 3], F32,
                              kind="ExternalOutput").ap()
    zc_dram = nc.dram_tensor("zc", [1, NU + 1], F32, kind="ExternalOutput").ap()

    with tile.TileContext(nc) as tc:
        with (
            tc.tile_pool(name="const", bufs=1) as const,
            tc.tile_pool(name="xpool", bufs=3) as xpool,
            tc.tile_pool(name="mpool", bufs=2) as mpool,
            tc.tile_pool(name="hpool", bufs=3) as hpool,
            tc.tile_pool(name="apool", bufs=2) as apool,
            tc.tile_pool(name="ppool", bufs=2) as ppool,
            tc.tile_pool(name="ptpool", bufs=2) as ptpool,
            tc.tile_pool(name="opool", bufs=2) as opool,
            tc.tile_pool(name="ps_e", bufs=2, space="PSUM") as ps_e,
            tc.tile_pool(name="ps_sc", bufs=1, space="PSUM") as ps_sc,
        ):
            # ---- constants first on the sync queue (tiny; x-loads follow) --
            vrep_sb = const.tile([f, 128], BF16)
            nc.sync.dma_start(vrep_sb, vrep_dram)
            wet_sb = const.tile([128, KD, f], BF16)
            nc.sync.dma_start(wet_sb, wet_dram.rearrange("p (k f) -> p k f", k=KD))
            vbb_sb = const.tile([128, 1], F32)
            nc.sync.dma_start(vbb_sb, vbb_dram)
            dvec_sb = const.tile([f, bp], F32)
            nc.sync.dma_start(dvec_sb, dvec_dram)
            zcols_sb = const.tile([128, NU + 1], F32)

            def load_x(u):
                pair, r0, nr, _, _, _ = UNITS[u]
                xt = xpool.tile([128, 2, KD, SC], BF16, tag="x", name=f"x_{u}")
                src = xt_dram[pair].rearrange("p (r k s) -> p r k s", r=2, k=KD)
                for rr in range(nr):
                    nc.sync.dma_start(xt[:, rr], src[:, r0 + rr])
                return xt

            xts = {u: load_x(u) for u in range(min(3, NU))}

            state = {}
            pts = {}
            ptas = {}

            def get_pt(b):
                if b not in pts:
                    pts[b] = ptpool.tile([128, NKF, NCH, TW], BF16, tag="pt",
                                         name=f"pt_{b}")
                    ptas[b] = ptpool.tile([128, KD - NKF, 3], F32, tag="pta",
                                          name=f"pta_{b}")
                return pts[b], ptas[b]

            def e_unit(u):
                nr = UNITS[u][2]
                xt = xts[u]
                es = []
                for rr in range(nr):
                    e_ps = ps_e.tile([f, SC], F32, tag="e", name=f"e_{u}_{rr}")
                    for k in range(KD):
                        for h2 in range(2):
                            sl = slice(h2 * 512, (h2 + 1) * 512)
                            nc.tensor.matmul(
                                e_ps[:, sl], wet_sb[:, k, :], xt[:, rr, k, sl],
                                start=(k == 0), stop=(k == KD - 1))
                    es.append(e_ps)
                state[u] = {"xt": xt, "e": es}

            def tanh_unit(u):
                b = UNITS[u][3]
                es = state[u].pop("e")
                hs = []
                for rr, e_ps in enumerate(es):
                    h_sb = hpool.tile([f, SC], BF16, tag="h", name=f"h_{u}_{rr}")
                    nc.scalar.activation(h_sb, e_ps, AF.Tanh,
                                         bias=dvec_sb[:, b:b + 1])
                    hs.append(h_sb)
                state[u]["h"] = hs

            def scores_unit(u):
                hs = state[u].pop("h")
                sc2 = ps_sc.tile([128, 2, SC], F32, tag="sc", name=f"sc_{u}")
                for rr, h_sb in enumerate(hs):
                    for h2 in range(2):
                        sl = slice(h2 * 512, (h2 + 1) * 512)
                        nc.tensor.matmul(sc2[:, rr, sl], vrep_sb, h_sb[:, sl],
                                         start=True, stop=True)
                state[u]["sc2"] = sc2

            def exp_unit(u):
                nr = UNITS[u][2]
                sc2 = state[u].pop("sc2")
                a2 = apool.tile([128, 2, SC], BF16, tag="a", name=f"a_{u}")
                nc.scalar.activation(
                    a2[:, 0:nr].rearrange("p r s -> p (r s)"),
                    sc2[:, 0:nr].rearrange("p r s -> p (r s)"),
                    AF.Exp, bias=vbb_sb, accum_out=zcols_sb[:, u:u + 1])
                state[u]["a2"] = a2

            def wsum_unit(u):
                _, _, nr, b, c0, _ = UNITS[u]
                st = state[u]
                xt, a2 = st.pop("xt"), st.pop("a2")
                xts.pop(u)
                pt, _ = get_pt(b)
                # DVE: one broadcast mult for all 8 k-slices of the unit
                # (last unit: ACT-reduced slices first so the drain overlaps)
                m2 = mpool.tile([128, 2, KD, SC], BF16, tag="m", name=f"m_{u}")
                ksls = ([slice(NKF, KD), slice(0, NKF)] if u == NU - 1
                        else [slice(0, KD)])
                for ksl in ksls:
                    nc.vector.tensor_tensor(
                        m2[:, 0:nr, ksl, :], xt[:, 0:nr, ksl, :],
                        a2[:, 0:nr].unsqueeze(2).broadcast_to(
                            [128, nr, ksl.stop - ksl.start, SC]),
                        op=ALU.mult)
                # DVE: in-place fold tree on slices 0..NKF-1 down to 2*TW
                h = SC // 2
                while h >= 2 * TW:
                    nc.vector.tensor_tensor(
                        m2[:, 0:nr, 0:NKF, 0:h], m2[:, 0:nr, 0:NKF, 0:h],
                        m2[:, 0:nr, 0:NKF, h:2 * h], op=ALU.add)
                    h //= 2
                # last fold drops tails into the per-batch partial tile
                nc.vector.tensor_tensor(
                    pt[:, :, c0:c0 + nr, :],
                    m2[:, 0:nr, 0:NKF, 0:TW].rearrange("p r k s -> p k r s"),
                    m2[:, 0:nr, 0:NKF, TW:2 * TW].rearrange("p r k s -> p k r s"),
                    op=ALU.add)
                st["m2"] = m2

            def act_reduces(u):
                _, _, nr, b, _, pcol = UNITS[u]
                m2 = state.pop(u)["m2"]
                _, pta = get_pt(b)
                # ACT: unit-wide copy+accum reduces for k=NKF..7 (chunks of
                # one batch summed together; fin() sums chunks anyway)
                da = ppool.tile([128, 2, SC], BF16, tag="da", name=f"da_{u}")
                for ki in range(NKF, KD):
                    nc.scalar.activation(
                        da[:, 0:nr], m2[:, 0:nr, ki, :], AF.Copy,
                        accum_out=pta[:, ki - NKF, pcol:pcol + 1])

            def fin(b):
                # raw partials go to the host; it does the tiny final sums
                pt, pta = pts.pop(b), ptas.pop(b)
                nc.gpsimd.dma_start(pt_dram[b],
                                    pt.rearrange("p k c s -> p (k c s)"))
                nc.gpsimd.dma_start(pta_dram[b],
                                    pta.rearrange("p k c -> p (k c)"))

            # ---- PE warmup + pipe priming while x_0 streams in: dummy bf16
            # matmuls open the HAM clock gate, and a garbage e->tanh->scores
            # ->exp chain fills every engine's pipeline so real unit 0 flows
            # through a hot pipe.
            warm_ps = ps_e.tile([f, SC], F32, tag="e", name="warm_ps")
            for w in range(40):
                nc.tensor.matmul(warm_ps[:, 0:128], vrep_sb, vrep_sb,
                                 start=(w == 0), stop=(w == 39))
            e_d = ps_e.tile([f, SC], F32, tag="e", name="e_d")
            for k in range(KD):
                nc.tensor.matmul(e_d[:, 0:128], wet_sb[:, 0, :],
                                 wet_sb[:, k, :], start=(k == 0),
                                 stop=(k == KD - 1))
            h_d = hpool.tile([f, SC], BF16, tag="h", name="h_d")
            nc.scalar.activation(h_d[:, 0:128], e_d[:, 0:128], AF.Tanh,
                                 bias=dvec_sb[:, 0:1])
            sc_d = ps_sc.tile([128, 2, SC], F32, tag="sc", name="sc_d")
            nc.tensor.matmul(sc_d[:, 0, 0:128], vrep_sb, h_d[:, 0:128],
                             start=True, stop=True)
            a_d = apool.tile([128, 2, SC], BF16, tag="a", name="a_d")
            nc.scalar.activation(a_d[:, 0, 0:128], sc_d[:, 0, 0:128],
                                 AF.Exp, bias=vbb_sb,
                                 accum_out=zcols_sb[:, NU:NU + 1])

            # ---- software-pipelined issue over units ----
            for i in range(NU + 4):
                if i + 3 < NU:
                    xts[i + 3] = load_x(i + 3)
                if 0 <= i - 2 < NU:
                    exp_unit(i - 2)
                if 0 <= i - 1 < NU:
                    tanh_unit(i - 1)
                    scores_unit(i - 1)
                if i < NU:
                    e_unit(i)
                if 0 <= i - 2 < NU:
                    wsum_unit(i - 2)
                if 0 <= i - 3 < NU:
                    act_reduces(i - 3)
                    b = UNITS[i - 3][3]
                    if LAST_UNIT_OF_BATCH[b] == i - 3:
                        fin(b)
            nc.gpsimd.dma_start(zc_dram, zcols_sb[0:1, :])

    nc.finalize()
    return nc


_NC_CACHE = {}


def _get_nc(key, **kw):
    if key not in _NC_CACHE:
        _NC_CACHE[key] = build_nc(**kw)
    return _NC_CACHE[key]


def make_in_maps(e_hiddens, d_hiddens, We_w, We_b, Wd_w, v_w, v_b, n_cores=N_CORES):
    bp = e_hiddens.shape[0] // n_cores
    bf16 = ml_dtypes.bfloat16

    def arrange(m):  # [D, x] -> [128, KD*x], partition-major tiles
        dd, xx = m.shape
        return np.ascontiguousarray(
            m.reshape(dd // 128, 128, xx).transpose(1, 0, 2).reshape(128, -1))

    wet = arrange(np.ascontiguousarray(We_w.T)).astype(bf16)    # [128, KD*F]
    vrep = np.ascontiguousarray(
        np.repeat(v_w[0][:, None], 128, axis=1)).astype(bf16)   # [F, 128]
    vbb = np.full((128, 1), np.float32(v_b[0]), np.float32)
    maps = []
    for i in range(n_cores):
        xc = e_hiddens[i * bp:(i + 1) * bp]                     # [bp, S, D]
        # xt[pair, p, r*KD*SC + k*SC + s'] = x[b, (2*half+r)*SC+s', k*128+p]
        xt = np.ascontiguousarray(
            xc.reshape(bp, 2, 2, SC, KD, 128).transpose(0, 1, 5, 2, 4, 3)
        ).astype(bf16).reshape(bp * 2, 128, 2 * KD * SC)
        # dvec[f, b] = Wd @ d_hiddens[b] + We_b (the tanh bias), tiny on host
        dvec = (d_hiddens[i * bp:(i + 1) * bp] @ Wd_w.T).T + We_b[:, None]
        maps.append({
            "xt": xt,
            "wet": wet,
            "vrep": vrep,
            "vbb": vbb,
            "dvec": np.ascontiguousarray(dvec, np.float32),
        })
    return maps


def kernel(e_hiddens, d_hiddens, length_mask, We_w, We_b, Wd_w, v_w, v_b,
           _trace=False):
    """Full inputs in, full output out.  length_mask is all-ones (the
    reference adds (1-mask)*1e-32, numerically a no-op)."""
    e_hiddens = np.asarray(e_hiddens, np.float32)
    d_hiddens = np.asarray(d_hiddens, np.float32)
    We_w = np.asarray(We_w, np.float32)
    We_b = np.asarray(We_b, np.float32)
    Wd_w = np.asarray(Wd_w, np.float32)
    v_w = np.asarray(v_w, np.float32)
    v_b = np.asarray(v_b, np.float32)

    nc = _get_nc("full")
    in_maps = make_in_maps(e_hiddens, d_hiddens, We_w, We_b, Wd_w, v_w, v_b)
    res = run_bass_kernel_spmd(nc, in_maps, list(range(N_CORES)), trace=_trace)
    outs = []
    for m in res.results:
        # pt[b, p, k, c, tw] bf16 fold tails; pta[b, p, k', col] f32
        pt = m["pt"].astype(np.float32).reshape(BP, 128, NKF, NCH, TW)
        pta = m["pta"].reshape(BP, 128, KD - NKF, 3)
        zc = m["zc"].reshape(-1)
        acc = np.empty((BP, 128, KD), np.float32)
        acc[:, :, 0:NKF] = pt.sum(axis=(3, 4))
        z = np.empty((BP, 1), np.float32)
        for b in range(BP):
            ncols = max(u[5] for u in UNITS if u[3] == b) + 1
            acc[b, :, NKF:KD] = pta[b, :, :, 0:ncols].sum(axis=2)
            z0, z1 = ZCOL0_OF_BATCH[b]
            z[b, 0] = zc[z0:z1].sum()
        o = acc.transpose(0, 2, 1).reshape(BP, D)  # [bp,p,k] -> [bp,d]
        outs.append(o / z)
    out = np.concatenate(outs, axis=0)
    if _trace:
        kernel.last_results = res
    return out
